# revision 5
# baseline (speedup 1.0000x reference)
"""GP posterior mean: mu = K_rbf(X_test, X_train) @ alpha on 8 NeuronCores.

Math: K[j,i] = sf2 * exp(-0.5*||xt_i - x_j||^2 / ell2).  The whole exponent is
expressed as a single dot product  exponent[j,i] = A[:,j] . B[:,i]  with a
14-long contraction built from bf16 hi/lo splits of the fp32 operands, so the
TensorE runs the distance matrix at full bf16 speed with ~fp32 accuracy.
ScalarE applies exp (sf2 folded into the activation bias), and a second
TensorE matmul contracts K against bf16 hi/lo-split alpha, accumulating in
PSUM over all train tiles.  Data-parallel over X_test rows: each of the 8
cores handles 2048 test points and needs no communication.
"""

import numpy as np
import ml_dtypes

M = 16384
N = 16384
NCORES = 8
MC = M // NCORES          # 2048 test points per core
CHUNK = 512               # test-chunk per PSUM bank
NCH = MC // CHUNK         # 4 chunks per core
NJT = N // 128            # 128 train tiles
C = 14                    # contraction length of the exponent matmul

_cache = {}


def _split2(v):
    hi = v.astype(ml_dtypes.bfloat16)
    lo = (v - hi.astype(np.float64)).astype(ml_dtypes.bfloat16)
    return hi, lo


def _split3(v):
    hi = v.astype(ml_dtypes.bfloat16)
    r = v - hi.astype(np.float64)
    mid = r.astype(ml_dtypes.bfloat16)
    lo = (r - mid.astype(np.float64)).astype(ml_dtypes.bfloat16)
    return hi, mid, lo


def _build_program(bias):
    import concourse.mybir as mybir
    import concourse.tile as tile
    from concourse import bacc

    fp32 = mybir.dt.float32
    bf16 = mybir.dt.bfloat16

    nc = bacc.Bacc(None, target_bir_lowering=False)
    A_d = nc.declare_dram_parameter("A", [C, N], bf16, isOutput=False)
    B_d = nc.declare_dram_parameter("B", [C, MC], bf16, isOutput=False)
    AL_d = nc.declare_dram_parameter("AL", [128, NJT * 4], bf16, isOutput=False)
    OUT_d = nc.declare_dram_parameter("out", [4, MC], fp32, isOutput=True)

    with tile.TileContext(nc) as tc:
        with (
            tc.tile_pool(name="singles", bufs=1) as singles,
            tc.tile_pool(name="kpool", bufs=4) as kpool,
            tc.tile_pool(name="opool", bufs=2) as opool,
            tc.tile_pool(name="pse", bufs=4, space="PSUM") as pse,
            tc.tile_pool(name="psacc", bufs=1, space="PSUM") as psacc,
        ):
            sb_A = singles.tile([C, N], bf16)
            for ch in range(8):
                s = slice(ch * (N // 8), (ch + 1) * (N // 8))
                nc.sync.dma_start(out=sb_A[:, s], in_=A_d[:, s])
            sb_B = singles.tile([C, MC], bf16)
            nc.sync.dma_start(out=sb_B, in_=B_d[:])
            sb_AL = singles.tile([128, NJT * 4], bf16)
            nc.sync.dma_start(out=sb_AL, in_=AL_d[:])
            accs = [
                psacc.tile([4, CHUNK], fp32, name=f"acc{i}") for i in range(NCH)
            ]
            for jt in range(NJT):
                ks = []
                for c in range(NCH):
                    e = pse.tile([128, CHUNK], fp32)
                    nc.tensor.matmul(
                        e,
                        lhsT=sb_A[:, jt * 128 : (jt + 1) * 128],
                        rhs=sb_B[:, c * CHUNK : (c + 1) * CHUNK],
                        start=True,
                        stop=True,
                    )
                    k = kpool.tile([128, CHUNK], bf16)
                    nc.scalar.activation(
                        k, e, mybir.ActivationFunctionType.Exp, bias=float(bias)
                    )
                    ks.append(k)
                for c in range(NCH):
                    nc.tensor.matmul(
                        accs[c],
                        lhsT=sb_AL[:, jt * 4 : (jt + 1) * 4],
                        rhs=ks[c],
                        start=(jt == 0),
                        stop=(jt == NJT - 1),
                    )
            for c in range(NCH):
                o = opool.tile([4, CHUNK], fp32)
                nc.vector.tensor_copy(o, accs[c])
                nc.sync.dma_start(
                    out=OUT_d[:, c * CHUNK : (c + 1) * CHUNK], in_=o
                )
    nc.compile()
    return nc


def _prep_inputs(X_test, X_train, alpha, log_lengthscale, log_outputscale):
    ell = np.exp(np.float32(log_lengthscale))
    ell2 = np.float64(np.float32(ell) ** 2)
    sf = np.exp(np.float32(log_outputscale))
    sf2 = np.float64(np.float32(sf) ** 2)

    xt = X_train.astype(np.float64)
    xs = X_test.astype(np.float64)
    al = alpha.astype(np.float64)

    # Train-side matrix A (C, N)
    x0h, x0l = _split2(xt[:, 0])
    x1h, x1l = _split2(xt[:, 1])
    pj = -(xt[:, 0] ** 2 + xt[:, 1] ** 2) / (2.0 * ell2)
    pjh, pjm, pjl = _split3(pj)
    ones = np.ones(N, dtype=ml_dtypes.bfloat16)
    A = np.stack(
        [ones, ones, ones, x0h, x0h, x0l, x0l, x1h, x1h, x1l, x1l, pjh, pjm, pjl]
    )

    # Test-side matrix B (C, M)
    T0 = -(xs[:, 0] ** 2 + xs[:, 1] ** 2) / (2.0 * ell2)
    T0h, T0m, T0l = _split3(T0)
    u0 = xs[:, 0] / ell2
    u0h, u0l = _split2(u0)
    u1 = xs[:, 1] / ell2
    u1h, u1l = _split2(u1)
    onesM = np.ones(M, dtype=ml_dtypes.bfloat16)
    B = np.stack(
        [T0h, T0m, T0l, u0h, u0l, u0h, u0l, u1h, u1l, u1h, u1l, onesM, onesM, onesM]
    )

    # alpha tiles (128, NJT*4): hi/lo split of each alpha column
    arh, arl = _split2(al[:, 0])
    aih, ail = _split2(al[:, 1])
    AL = np.stack([arh, arl, aih, ail], axis=1)  # (N, 4)
    AL = AL.reshape(NJT, 128, 4).transpose(1, 0, 2).reshape(128, NJT * 4)
    AL = np.ascontiguousarray(AL)

    bias = np.float32(np.log(sf2))
    return A, B, AL, bias


def kernel(X_test, X_train, alpha, log_lengthscale, log_outputscale):
    from concourse.bass_utils import run_bass_kernel_spmd

    A, B, AL, bias = _prep_inputs(
        X_test, X_train, alpha, log_lengthscale, log_outputscale
    )

    key = ("nc", float(bias))
    if key not in _cache:
        _cache[key] = _build_program(bias)
    nc = _cache[key]

    core_ids = list(range(NCORES))
    in_maps = []
    for c in core_ids:
        in_maps.append(
            {
                "A": A,
                "B": np.ascontiguousarray(B[:, c * MC : (c + 1) * MC]),
                "AL": AL,
            }
        )
    res = run_bass_kernel_spmd(nc, in_maps, core_ids)

    out = np.empty((M, 2), dtype=np.float32)
    for c in core_ids:
        o = res.results[c]["out"]
        out[c * MC : (c + 1) * MC, 0] = o[0] + o[1]
        out[c * MC : (c + 1) * MC, 1] = o[2] + o[3]
    return out


# revision 6
# speedup vs baseline: 1.6679x; 1.6679x over previous
"""GP posterior mean: mu = K_rbf(X_test, X_train) @ alpha on 8 NeuronCores.

Math: K[j,i] = sf2 * exp(-0.5*||xt_i - x_j||^2 / ell2).  The whole exponent is
expressed as a single dot product  exponent[j,i] = A[:,j] . B[:,i]  with a
14-long contraction built from bf16 hi/lo splits of the fp32 operands, so the
TensorE runs the distance matrix at full bf16 speed with ~fp32 accuracy.
The contraction is zero-padded to 128: sub-128 contractions keep the PE at
the throttled 1.2 GHz clock (only quadrant 0 active), while K=128 streams at
2.4 GHz.  ScalarE applies exp (sf2 folded into the activation bias), and a
second TensorE matmul contracts K against bf16 hi/lo-split alpha,
accumulating in PSUM over all train tiles.  Data-parallel over X_test rows:
each of the 8 cores handles 2048 test points with no communication.
"""

import numpy as np
import ml_dtypes

M = 16384
N = 16384
NCORES = 8
MC = M // NCORES          # 2048 test points per core
CHUNK = 1024              # test-chunk per ACT instruction (2 PSUM banks)
NCH = MC // CHUNK         # 2 chunks per core
NJT = N // 128            # 128 train tiles
C = 14                    # used contraction rows of the exponent matmul
CP = 128                  # padded contraction (keeps PE at full clock)

_cache = {}


def _split2(v):
    hi = v.astype(ml_dtypes.bfloat16)
    lo = (v - hi.astype(np.float64)).astype(ml_dtypes.bfloat16)
    return hi, lo


def _split3(v):
    hi = v.astype(ml_dtypes.bfloat16)
    r = v - hi.astype(np.float64)
    mid = r.astype(ml_dtypes.bfloat16)
    lo = (r - mid.astype(np.float64)).astype(ml_dtypes.bfloat16)
    return hi, mid, lo


def _build_program(bias):
    import concourse.mybir as mybir
    import concourse.tile as tile
    from concourse import bacc

    fp32 = mybir.dt.float32
    bf16 = mybir.dt.bfloat16

    nc = bacc.Bacc(None, target_bir_lowering=False)
    A_d = nc.declare_dram_parameter("A", [CP, N], bf16, isOutput=False)
    B_d = nc.declare_dram_parameter("B", [CP, MC], bf16, isOutput=False)
    AL_d = nc.declare_dram_parameter("AL", [128, NJT * 4], bf16, isOutput=False)
    OUT_d = nc.declare_dram_parameter("out", [4, MC], fp32, isOutput=True)

    with tile.TileContext(nc) as tc:
        with (
            tc.tile_pool(name="singles", bufs=1) as singles,
            tc.tile_pool(name="kpool", bufs=3) as kpool,
            tc.tile_pool(name="opool", bufs=2) as opool,
            tc.tile_pool(name="pse", bufs=2, space="PSUM") as pse,
            tc.tile_pool(name="psacc", bufs=1, space="PSUM") as psacc,
        ):
            sb_A = singles.tile([CP, N], bf16)
            for ch in range(16):
                s = slice(ch * (N // 16), (ch + 1) * (N // 16))
                nc.sync.dma_start(out=sb_A[:, s], in_=A_d[:, s])
            sb_B = singles.tile([CP, MC], bf16)
            nc.sync.dma_start(out=sb_B, in_=B_d[:])
            sb_AL = singles.tile([128, NJT * 4], bf16)
            nc.sync.dma_start(out=sb_AL, in_=AL_d[:])
            accs = [
                psacc.tile([4, CHUNK], fp32, name=f"acc{i}") for i in range(NCH)
            ]
            for jt in range(NJT):
                for c in range(NCH):
                    e = pse.tile([128, CHUNK], fp32)
                    for h in range(CHUNK // 512):
                        nc.tensor.matmul(
                            e[:, h * 512 : (h + 1) * 512],
                            lhsT=sb_A[:, jt * 128 : (jt + 1) * 128],
                            rhs=sb_B[
                                :, c * CHUNK + h * 512 : c * CHUNK + (h + 1) * 512
                            ],
                            start=True,
                            stop=True,
                        )
                    k = kpool.tile([128, CHUNK], bf16)
                    nc.scalar.activation(
                        k, e, mybir.ActivationFunctionType.Exp, bias=float(bias)
                    )
                    for h in range(CHUNK // 512):
                        nc.tensor.matmul(
                            accs[c][:, h * 512 : (h + 1) * 512],
                            lhsT=sb_AL[:, jt * 4 : (jt + 1) * 4],
                            rhs=k[:, h * 512 : (h + 1) * 512],
                            start=(jt == 0),
                            stop=(jt == NJT - 1),
                        )
            for c in range(NCH):
                o = opool.tile([4, CHUNK], fp32, name=f"o{c}")
                nc.vector.tensor_copy(o, accs[c])
                nc.sync.dma_start(
                    out=OUT_d[:, c * CHUNK : (c + 1) * CHUNK], in_=o
                )
    nc.compile()
    return nc


def _prep_inputs(X_test, X_train, alpha, log_lengthscale, log_outputscale):
    ell = np.exp(np.float32(log_lengthscale))
    ell2 = np.float64(np.float32(ell) ** 2)
    sf = np.exp(np.float32(log_outputscale))
    sf2 = np.float64(np.float32(sf) ** 2)

    xt = X_train.astype(np.float64)
    xs = X_test.astype(np.float64)
    al = alpha.astype(np.float64)

    # Train-side matrix A (CP, N); rows 14.. are zero padding
    x0h, x0l = _split2(xt[:, 0])
    x1h, x1l = _split2(xt[:, 1])
    pj = -(xt[:, 0] ** 2 + xt[:, 1] ** 2) / (2.0 * ell2)
    pjh, pjm, pjl = _split3(pj)
    ones = np.ones(N, dtype=ml_dtypes.bfloat16)
    A = np.zeros((CP, N), dtype=ml_dtypes.bfloat16)
    A[:C] = np.stack(
        [ones, ones, ones, x0h, x0h, x0l, x0l, x1h, x1h, x1l, x1l, pjh, pjm, pjl]
    )

    # Test-side matrix B (CP, M); rows 14.. are zero padding
    T0 = -(xs[:, 0] ** 2 + xs[:, 1] ** 2) / (2.0 * ell2)
    T0h, T0m, T0l = _split3(T0)
    u0 = xs[:, 0] / ell2
    u0h, u0l = _split2(u0)
    u1 = xs[:, 1] / ell2
    u1h, u1l = _split2(u1)
    onesM = np.ones(M, dtype=ml_dtypes.bfloat16)
    B = np.zeros((CP, M), dtype=ml_dtypes.bfloat16)
    B[:C] = np.stack(
        [T0h, T0m, T0l, u0h, u0l, u0h, u0l, u1h, u1l, u1h, u1l, onesM, onesM, onesM]
    )

    # alpha tiles (128, NJT*4): hi/lo split of each alpha column
    arh, arl = _split2(al[:, 0])
    aih, ail = _split2(al[:, 1])
    AL = np.stack([arh, arl, aih, ail], axis=1)  # (N, 4)
    AL = AL.reshape(NJT, 128, 4).transpose(1, 0, 2).reshape(128, NJT * 4)
    AL = np.ascontiguousarray(AL)

    bias = np.float32(np.log(sf2))
    return A, B, AL, bias


def kernel(X_test, X_train, alpha, log_lengthscale, log_outputscale):
    from concourse.bass_utils import run_bass_kernel_spmd

    A, B, AL, bias = _prep_inputs(
        X_test, X_train, alpha, log_lengthscale, log_outputscale
    )

    key = ("nc", float(bias))
    if key not in _cache:
        _cache[key] = _build_program(bias)
    nc = _cache[key]

    core_ids = list(range(NCORES))
    in_maps = []
    for c in core_ids:
        in_maps.append(
            {
                "A": A,
                "B": np.ascontiguousarray(B[:, c * MC : (c + 1) * MC]),
                "AL": AL,
            }
        )
    res = run_bass_kernel_spmd(nc, in_maps, core_ids)

    out = np.empty((M, 2), dtype=np.float32)
    for c in core_ids:
        o = res.results[c]["out"]
        out[c * MC : (c + 1) * MC, 0] = o[0] + o[1]
        out[c * MC : (c + 1) * MC, 1] = o[2] + o[3]
    return out


# revision 7
# speedup vs baseline: 1.7284x; 1.0362x over previous
"""GP posterior mean: mu = K_rbf(X_test, X_train) @ alpha on 8 NeuronCores.

Math: K[j,i] = sf2 * exp(-0.5*||xt_i - x_j||^2 / ell2).  The whole exponent is
expressed as a single dot product  exponent[j,i] = A[:,j] . B[:,i]  with a
14-long contraction built from bf16 hi/lo splits of the fp32 operands, so the
TensorE runs the distance matrix at full bf16 speed with ~fp32 accuracy.
The contraction is zero-padded to 128: sub-128 contractions keep the PE at
the throttled 1.2 GHz clock (only quadrant 0 active), while K=128 streams at
2.4 GHz.  ScalarE applies exp (sf2 folded into the activation bias), and a
second TensorE matmul contracts K against bf16 hi/lo-split alpha,
accumulating in PSUM over all train tiles.  Data-parallel over X_test rows:
each of the 8 cores handles 2048 test points with no communication.
"""

import numpy as np
import ml_dtypes

M = 16384
N = 16384
NCORES = 8
MC = M // NCORES          # 2048 test points per core
CHUNK = 1024              # test-chunk per ACT instruction (2 PSUM banks)
NCH = MC // CHUNK         # 2 chunks per core
NJT = N // 128            # 128 train tiles
C = 14                    # used contraction rows of the exponent matmul
CP = 128                  # padded contraction (keeps PE at full clock)

_cache = {}


def _split2(v):
    hi = v.astype(ml_dtypes.bfloat16)
    lo = (v - hi.astype(np.float64)).astype(ml_dtypes.bfloat16)
    return hi, lo


def _split3(v):
    hi = v.astype(ml_dtypes.bfloat16)
    r = v - hi.astype(np.float64)
    mid = r.astype(ml_dtypes.bfloat16)
    lo = (r - mid.astype(np.float64)).astype(ml_dtypes.bfloat16)
    return hi, mid, lo


def _build_program(bias):
    import concourse.mybir as mybir
    import concourse.tile as tile
    from concourse import bacc

    fp32 = mybir.dt.float32
    bf16 = mybir.dt.bfloat16

    nc = bacc.Bacc(None, target_bir_lowering=False)
    A_d = nc.declare_dram_parameter("A", [CP, N], bf16, isOutput=False)
    B_d = nc.declare_dram_parameter("B", [CP, MC], bf16, isOutput=False)
    AL_d = nc.declare_dram_parameter("AL", [128, NJT * 4], bf16, isOutput=False)
    OUT_d = nc.declare_dram_parameter("out", [4, MC], fp32, isOutput=True)

    with tile.TileContext(nc) as tc:
        with (
            tc.tile_pool(name="singles", bufs=1) as singles,
            tc.tile_pool(name="kpool", bufs=6) as kpool,
            tc.tile_pool(name="opool", bufs=2) as opool,
            tc.tile_pool(name="pse", bufs=2, space="PSUM") as pse,
            tc.tile_pool(name="psacc", bufs=1, space="PSUM") as psacc,
        ):
            sb_B = singles.tile([CP, MC], bf16)
            nc.sync.dma_start(out=sb_B, in_=B_d[:])
            sb_AL = singles.tile([128, NJT * 4], bf16)
            nc.gpsimd.dma_start(out=sb_AL, in_=AL_d[:])
            sb_A = singles.tile([CP, N], bf16)
            for ch in range(32):
                s = slice(ch * (N // 32), (ch + 1) * (N // 32))
                eng = nc.sync if ch % 2 == 0 else nc.gpsimd
                eng.dma_start(out=sb_A[:, s], in_=A_d[:, s])
            accs = [
                psacc.tile([4, CHUNK], fp32, name=f"acc{i}") for i in range(NCH)
            ]
            for jt in range(NJT):
                for c in range(NCH):
                    e = pse.tile([128, CHUNK], fp32)
                    for h in range(CHUNK // 512):
                        nc.tensor.matmul(
                            e[:, h * 512 : (h + 1) * 512],
                            lhsT=sb_A[:, jt * 128 : (jt + 1) * 128],
                            rhs=sb_B[
                                :, c * CHUNK + h * 512 : c * CHUNK + (h + 1) * 512
                            ],
                            start=True,
                            stop=True,
                        )
                    k = kpool.tile([128, CHUNK], bf16)
                    nc.scalar.activation(
                        k, e, mybir.ActivationFunctionType.Exp, bias=float(bias)
                    )
                    for h in range(CHUNK // 512):
                        nc.tensor.matmul(
                            accs[c][:, h * 512 : (h + 1) * 512],
                            lhsT=sb_AL[:, jt * 4 : (jt + 1) * 4],
                            rhs=k[:, h * 512 : (h + 1) * 512],
                            start=(jt == 0),
                            stop=(jt == NJT - 1),
                        )
            for c in range(NCH):
                o = opool.tile([4, CHUNK], fp32, name=f"o{c}")
                nc.vector.tensor_copy(o, accs[c])
                nc.sync.dma_start(
                    out=OUT_d[:, c * CHUNK : (c + 1) * CHUNK], in_=o
                )
    nc.compile()
    return nc


def _prep_inputs(X_test, X_train, alpha, log_lengthscale, log_outputscale):
    ell = np.exp(np.float32(log_lengthscale))
    ell2 = np.float64(np.float32(ell) ** 2)
    sf = np.exp(np.float32(log_outputscale))
    sf2 = np.float64(np.float32(sf) ** 2)

    xt = X_train.astype(np.float64)
    xs = X_test.astype(np.float64)
    al = alpha.astype(np.float64)

    # Train-side matrix A (CP, N); rows 14.. are zero padding
    x0h, x0l = _split2(xt[:, 0])
    x1h, x1l = _split2(xt[:, 1])
    pj = -(xt[:, 0] ** 2 + xt[:, 1] ** 2) / (2.0 * ell2)
    pjh, pjm, pjl = _split3(pj)
    ones = np.ones(N, dtype=ml_dtypes.bfloat16)
    A = np.zeros((CP, N), dtype=ml_dtypes.bfloat16)
    A[:C] = np.stack(
        [ones, ones, ones, x0h, x0h, x0l, x0l, x1h, x1h, x1l, x1l, pjh, pjm, pjl]
    )

    # Test-side matrix B (CP, M); rows 14.. are zero padding
    T0 = -(xs[:, 0] ** 2 + xs[:, 1] ** 2) / (2.0 * ell2)
    T0h, T0m, T0l = _split3(T0)
    u0 = xs[:, 0] / ell2
    u0h, u0l = _split2(u0)
    u1 = xs[:, 1] / ell2
    u1h, u1l = _split2(u1)
    onesM = np.ones(M, dtype=ml_dtypes.bfloat16)
    B = np.zeros((CP, M), dtype=ml_dtypes.bfloat16)
    B[:C] = np.stack(
        [T0h, T0m, T0l, u0h, u0l, u0h, u0l, u1h, u1l, u1h, u1l, onesM, onesM, onesM]
    )

    # alpha tiles (128, NJT*4): hi/lo split of each alpha column
    arh, arl = _split2(al[:, 0])
    aih, ail = _split2(al[:, 1])
    AL = np.stack([arh, arl, aih, ail], axis=1)  # (N, 4)
    AL = AL.reshape(NJT, 128, 4).transpose(1, 0, 2).reshape(128, NJT * 4)
    AL = np.ascontiguousarray(AL)

    bias = np.float32(np.log(sf2))
    return A, B, AL, bias


def kernel(X_test, X_train, alpha, log_lengthscale, log_outputscale):
    from concourse.bass_utils import run_bass_kernel_spmd

    A, B, AL, bias = _prep_inputs(
        X_test, X_train, alpha, log_lengthscale, log_outputscale
    )

    key = ("nc", float(bias))
    if key not in _cache:
        _cache[key] = _build_program(bias)
    nc = _cache[key]

    core_ids = list(range(NCORES))
    in_maps = []
    for c in core_ids:
        in_maps.append(
            {
                "A": A,
                "B": np.ascontiguousarray(B[:, c * MC : (c + 1) * MC]),
                "AL": AL,
            }
        )
    res = run_bass_kernel_spmd(nc, in_maps, core_ids)

    out = np.empty((M, 2), dtype=np.float32)
    for c in core_ids:
        o = res.results[c]["out"]
        out[c * MC : (c + 1) * MC, 0] = o[0] + o[1]
        out[c * MC : (c + 1) * MC, 1] = o[2] + o[3]
    return out


# revision 8
# speedup vs baseline: 1.9157x; 1.1084x over previous
"""GP posterior mean: mu = K_rbf(X_test, X_train) @ alpha on 8 NeuronCores.

Math: K[j,i] = sf2 * exp(-0.5*||xt_i - x_j||^2 / ell2).  The whole exponent is
expressed as a single dot product  exponent[j,i] = A[:,j] . B[:,i]  with a
14-long contraction built from bf16 hi/lo splits of the fp32 operands, so the
TensorE runs the distance matrix at full bf16 speed with ~fp32 accuracy.
The contraction is zero-padded to 128: sub-128 contractions keep the PE at
the throttled 1.2 GHz clock (only quadrant 0 active), while K=128 streams at
2.4 GHz.  ScalarE applies exp (sf2 folded into the activation bias), and a
second TensorE matmul contracts K against bf16 hi/lo-split alpha,
accumulating in PSUM over all train tiles.  Data-parallel over X_test rows:
each of the 8 cores handles 2048 test points with no communication.
"""

import numpy as np
import ml_dtypes

M = 16384
N = 16384
NCORES = 8
MC = M // NCORES          # 2048 test points per core
CHUNK = 1024              # test-chunk per ACT instruction (2 PSUM banks)
NCH = MC // CHUNK         # 2 chunks per core
NJT = N // 128            # 128 train tiles
C = 14                    # used contraction rows of the exponent matmul
CP = 128                  # padded contraction (keeps PE at full clock)

_cache = {}


def _split2(v):
    hi = v.astype(ml_dtypes.bfloat16)
    lo = (v - hi.astype(np.float64)).astype(ml_dtypes.bfloat16)
    return hi, lo


def _split3(v):
    hi = v.astype(ml_dtypes.bfloat16)
    r = v - hi.astype(np.float64)
    mid = r.astype(ml_dtypes.bfloat16)
    lo = (r - mid.astype(np.float64)).astype(ml_dtypes.bfloat16)
    return hi, mid, lo


def _build_program(bias):
    import concourse.mybir as mybir
    import concourse.tile as tile
    from concourse import bacc

    fp32 = mybir.dt.float32
    bf16 = mybir.dt.bfloat16

    nc = bacc.Bacc(None, target_bir_lowering=False)
    A_d = nc.declare_dram_parameter("A", [CP, N], bf16, isOutput=False)
    B_d = nc.declare_dram_parameter("B", [CP, MC], bf16, isOutput=False)
    AL_d = nc.declare_dram_parameter("AL", [128, NJT * 4], bf16, isOutput=False)
    OUT_d = nc.declare_dram_parameter("out", [4, MC], fp32, isOutput=True)

    with tile.TileContext(nc) as tc:
        with (
            tc.tile_pool(name="singles", bufs=1) as singles,
            tc.tile_pool(name="kpool", bufs=6) as kpool,
            tc.tile_pool(name="opool", bufs=2) as opool,
            tc.tile_pool(name="pse", bufs=3, space="PSUM") as pse,
            tc.tile_pool(name="psacc", bufs=1, space="PSUM") as psacc,
        ):
            sb_B = singles.tile([CP, MC], bf16)
            nc.sync.dma_start(out=sb_B, in_=B_d[:])
            sb_AL = singles.tile([128, NJT * 4], bf16)
            nc.gpsimd.dma_start(out=sb_AL, in_=AL_d[:])
            sb_A = singles.tile([CP, N], bf16)
            for ch in range(32):
                s = slice(ch * (N // 32), (ch + 1) * (N // 32))
                eng = nc.sync if ch % 2 == 0 else nc.gpsimd
                eng.dma_start(out=sb_A[:, s], in_=A_d[:, s])
            acc_all = psacc.tile([36, CHUNK], fp32, name="acc_all")
            accs = [acc_all[32 * i : 32 * i + 4, :] for i in range(NCH)]
            for jt in range(NJT):
                for c in range(NCH):
                    e = pse.tile([128, CHUNK], fp32)
                    for h in range(CHUNK // 512):
                        nc.tensor.matmul(
                            e[:, h * 512 : (h + 1) * 512],
                            lhsT=sb_A[:, jt * 128 : (jt + 1) * 128],
                            rhs=sb_B[
                                :, c * CHUNK + h * 512 : c * CHUNK + (h + 1) * 512
                            ],
                            start=True,
                            stop=True,
                        )
                    k = kpool.tile([128, CHUNK], bf16)
                    nc.scalar.activation(
                        k, e, mybir.ActivationFunctionType.Exp, bias=float(bias)
                    )
                    for h in range(CHUNK // 512):
                        nc.tensor.matmul(
                            accs[c][:, h * 512 : (h + 1) * 512],
                            lhsT=sb_AL[:, jt * 4 : (jt + 1) * 4],
                            rhs=k[:, h * 512 : (h + 1) * 512],
                            start=(jt == 0),
                            stop=(jt == NJT - 1),
                        )
            for c in range(NCH):
                o = opool.tile([4, CHUNK], fp32, name=f"o{c}")
                nc.vector.tensor_copy(o, accs[c])
                nc.sync.dma_start(
                    out=OUT_d[:, c * CHUNK : (c + 1) * CHUNK], in_=o
                )
    nc.compile()
    return nc


def _prep_inputs(X_test, X_train, alpha, log_lengthscale, log_outputscale):
    ell = np.exp(np.float32(log_lengthscale))
    ell2 = np.float64(np.float32(ell) ** 2)
    sf = np.exp(np.float32(log_outputscale))
    sf2 = np.float64(np.float32(sf) ** 2)

    xt = X_train.astype(np.float64)
    xs = X_test.astype(np.float64)
    al = alpha.astype(np.float64)

    # Train-side matrix A (CP, N); rows 14.. are zero padding
    x0h, x0l = _split2(xt[:, 0])
    x1h, x1l = _split2(xt[:, 1])
    pj = -(xt[:, 0] ** 2 + xt[:, 1] ** 2) / (2.0 * ell2)
    pjh, pjm, pjl = _split3(pj)
    ones = np.ones(N, dtype=ml_dtypes.bfloat16)
    A = np.zeros((CP, N), dtype=ml_dtypes.bfloat16)
    A[:C] = np.stack(
        [ones, ones, ones, x0h, x0h, x0l, x0l, x1h, x1h, x1l, x1l, pjh, pjm, pjl]
    )

    # Test-side matrix B (CP, M); rows 14.. are zero padding
    T0 = -(xs[:, 0] ** 2 + xs[:, 1] ** 2) / (2.0 * ell2)
    T0h, T0m, T0l = _split3(T0)
    u0 = xs[:, 0] / ell2
    u0h, u0l = _split2(u0)
    u1 = xs[:, 1] / ell2
    u1h, u1l = _split2(u1)
    onesM = np.ones(M, dtype=ml_dtypes.bfloat16)
    B = np.zeros((CP, M), dtype=ml_dtypes.bfloat16)
    B[:C] = np.stack(
        [T0h, T0m, T0l, u0h, u0l, u0h, u0l, u1h, u1l, u1h, u1l, onesM, onesM, onesM]
    )

    # alpha tiles (128, NJT*4): hi/lo split of each alpha column
    arh, arl = _split2(al[:, 0])
    aih, ail = _split2(al[:, 1])
    AL = np.stack([arh, arl, aih, ail], axis=1)  # (N, 4)
    AL = AL.reshape(NJT, 128, 4).transpose(1, 0, 2).reshape(128, NJT * 4)
    AL = np.ascontiguousarray(AL)

    bias = np.float32(np.log(sf2))
    return A, B, AL, bias


def kernel(X_test, X_train, alpha, log_lengthscale, log_outputscale):
    from concourse.bass_utils import run_bass_kernel_spmd

    A, B, AL, bias = _prep_inputs(
        X_test, X_train, alpha, log_lengthscale, log_outputscale
    )

    key = ("nc", float(bias))
    if key not in _cache:
        _cache[key] = _build_program(bias)
    nc = _cache[key]

    core_ids = list(range(NCORES))
    in_maps = []
    for c in core_ids:
        in_maps.append(
            {
                "A": A,
                "B": np.ascontiguousarray(B[:, c * MC : (c + 1) * MC]),
                "AL": AL,
            }
        )
    res = run_bass_kernel_spmd(nc, in_maps, core_ids)

    out = np.empty((M, 2), dtype=np.float32)
    for c in core_ids:
        o = res.results[c]["out"]
        out[c * MC : (c + 1) * MC, 0] = o[0] + o[1]
        out[c * MC : (c + 1) * MC, 1] = o[2] + o[3]
    return out
